# revision 18
# baseline (speedup 1.0000x reference)
"""Trainium2 Bass kernel for a dense transformer layer (AxialRoPE + cosine-sim
attention + SwiGLU FFN), distributed over 8 NeuronCores.

Sharding: core = 4*b + c handles batch b (of 2) and query-row chunk c (of 4,
512 rows each). Each core computes q/k/v for its own 512 rows; k (transposed,
rope'd, bf16) and v are AllGathered across the 4-core batch group so each core
attends over the full 2048-token sequence.

All matmuls in bf16; softmax/norms in f32.
"""

import math
import os

import numpy as np

D = 1024
L = 2048
NH = 16
DH = 64
DFF = 3072
EPS = 1e-6
CH = 512           # query rows per core
NT_KV = L // 128   # 16 l-tiles of gathered k/v
NT_Q = CH // 128   # 4 l-tiles of the own chunk
N_CORES = 8
RG = [[0, 1, 2, 3], [4, 5, 6, 7]]

_CACHE = {}
_PH = int(os.environ.get("BASS_KERNEL_PHASES", "6"))


def _build_nc():
    import concourse.bass as bass
    import concourse.mybir as mybir
    import concourse.tile as tile
    from concourse import bacc
    from concourse.masks import make_identity

    f32 = mybir.dt.float32
    bf16 = mybir.dt.bfloat16
    AF = mybir.ActivationFunctionType
    AX = mybir.AxisListType

    nc = bacc.Bacc("TRN2", target_bir_lowering=False, debug=False,
                   num_devices=N_CORES)

    x_q = nc.dram_tensor("x_q", [CH, D], f32, kind="ExternalInput").ap()
    pos_q = nc.dram_tensor("pos_q", [CH, 2], f32, kind="ExternalInput").ap()
    wqkvT = nc.dram_tensor("wqkvT", [D, 3 * D], bf16, kind="ExternalInput").ap()
    woT = nc.dram_tensor("woT", [D, D], bf16, kind="ExternalInput").ap()
    wupT = nc.dram_tensor("wupT", [D, 2 * DFF], bf16, kind="ExternalInput").ap()
    wdownT = nc.dram_tensor("wdownT", [DFF, D], bf16, kind="ExternalInput").ap()
    freqs_c = nc.dram_tensor("freqs_c", [1, 256], f32, kind="ExternalInput").ap()
    sqrtsc_c = nc.dram_tensor("sqrtsc_c", [1, 16], f32, kind="ExternalInput").ap()
    y = nc.dram_tensor("y", [CH, D], f32, kind="ExternalOutput").ap()

    # collective bounce buffers (internal DRAM); k and v gathered separately
    # so the k AllGather launches early and overlaps the rest of phase 2
    cc_k_in = nc.dram_tensor("cc_k_in", [512, D], bf16).ap()
    cc_k_out = nc.dram_tensor("cc_k_out", [2048, D], bf16).ap()
    cc_v_in = nc.dram_tensor("cc_v_in", [512, D], bf16).ap()
    cc_v_out = nc.dram_tensor("cc_v_out", [2048, D], bf16).ap()

    with tile.TileContext(nc) as tc:
        with (
            tc.tile_pool(name="const", bufs=1) as constp,
            tc.tile_pool(name="w3072", bufs=8) as w3072,
            tc.tile_pool(name="oT", bufs=16) as oTp,
            tc.tile_pool(name="ptrans", bufs=2, space="PSUM") as ptrans,
            tc.tile_pool(name="pmm", bufs=5, space="PSUM") as pmm,
            tc.tile_pool(name="pv", bufs=1, space="PSUM") as pvp,
        ):
            ident = constp.tile([128, 128], bf16)
            make_identity(nc, ident)
            freqs_rep = constp.tile([128, 256], f32)
            nc.sync.dma_start(out=freqs_rep, in_=freqs_c.broadcast_to([128, 256]))
            sqrtsc_rep = constp.tile([128, 16], f32)
            nc.sync.dma_start(out=sqrtsc_rep, in_=sqrtsc_c.broadcast_to([128, 16]))
            eps_c = constp.tile([128, 1], f32)
            nc.gpsimd.memset(eps_c, EPS)

            # qkv weights resident for phase 2
            wq = [w3072.tile([128, 3 * D], bf16, tag="w3072", name=f"wq{i}")
                  for i in range(8)]
            for k8 in range(8):
                nc.sync.dma_start(out=wq[k8], in_=wqkvT[k8 * 128:(k8 + 1) * 128, :])

            # attention output (transposed), one base-0 tile per head
            oT = [oTp.tile([128, CH], bf16, tag="oT", name=f"oT{i}")
                  for i in range(8)]

            def sincos(pool_sc, pool_tmp, pos_ap, i):
                """cos/sin tiles [128, 512] (16 heads x 32) for l-tile i."""
                pt = pool_tmp.tile([128, 2], f32, tag="pos")
                nc.sync.dma_start(out=pt, in_=pos_ap[i * 128:(i + 1) * 128, :])
                theta = pool_tmp.tile([128, 512], f32, tag="theta")
                tv = theta.rearrange("p (h t) -> p h t", h=16)
                fv = freqs_rep.rearrange("p (h t) -> p h t", h=16)
                nc.vector.tensor_scalar_mul(tv[:, :, 0:16], in0=fv, scalar1=pt[:, 0:1])
                nc.vector.tensor_scalar_mul(tv[:, :, 16:32], in0=fv, scalar1=pt[:, 1:2])
                # range-reduce into [-pi, pi] (ACT Sin domain):
                # k ~ theta/2pi (any rounding), theta -= k*2pi, one +-2pi wrap
                thk = pool_tmp.tile([128, 512], f32, tag="thk")
                nc.vector.tensor_scalar_mul(thk, in0=theta,
                                            scalar1=1.0 / (2 * math.pi))
                ki32 = pool_tmp.tile([128, 512], mybir.dt.int32, tag="thk")
                nc.vector.tensor_copy(out=ki32, in_=thk)
                kf = pool_tmp.tile([128, 512], f32, tag="thk")
                nc.vector.tensor_copy(out=kf, in_=ki32)
                c1 = float(np.float32(2 * math.pi))
                c2 = float(2 * math.pi - np.float64(np.float32(2 * math.pi)))
                nc.vector.cody_waite_cascade(out=theta, x=theta, k=kf,
                                             c1=c1, c2=c2, c3=0.0)
                nc.vector.add_range_wrap(out=theta, in_=theta, shift=0.0,
                                         bound=math.pi, period=2 * math.pi)
                cos_t = pool_sc.tile([128, 512], bf16, tag="sincos")
                sin_t = pool_sc.tile([128, 512], bf16, tag="sincos")
                nc.scalar.activation(sin_t, theta, AF.Sin)
                nc.vector.add_range_wrap(out=theta, in_=theta, shift=math.pi / 2,
                                         bound=math.pi, period=2 * math.pi)
                nc.scalar.activation(cos_t, theta, AF.Sin)
                return cos_t, sin_t

            def rms_norm_bf16(pool_tmp, xa):
                """xa [128, 1024] f32 -> h bf16 [128, 1024]."""
                sq = pool_tmp.tile([128, D], bf16, tag="sq")
                nc.vector.tensor_mul(sq, in0=xa, in1=xa)
                ssum = pool_tmp.tile([128, 1], f32, tag="ssum")
                nc.vector.reduce_sum(out=ssum, in_=sq, axis=AX.X)
                rstd = pool_tmp.tile([128, 1], f32, tag="rstd")
                nc.scalar.activation(rstd, ssum, AF.Sqrt, bias=eps_c, scale=1.0 / D)
                nc.vector.reciprocal(rstd, rstd)
                hb = pool_tmp.tile([128, D], bf16, tag="hb")
                nc.vector.tensor_scalar_mul(hb, in0=xa, scalar1=rstd)
                return hb

            def normrope(pool_tmp, kn, ch, cos_t, sin_t, rot_out):
                """cosine-normalize + rope heads ch*8..ch*8+8.

                kn: [128, 512] f32 PSUM (8 heads x 64); rot_out bf16 SBUF."""
                knv = kn.rearrange("p (h d) -> p h d", h=8)
                sq = pool_tmp.tile([128, 512], bf16, tag="nr_sq")
                sqv = sq.rearrange("p (h d) -> p h d", h=8)
                nc.scalar.square(sq, kn)
                ss = pool_tmp.tile([128, 8], f32, tag="nr_ss")
                nc.vector.reduce_sum(out=ss, in_=sqv, axis=AX.X)
                nc.scalar.activation(ss, ss, AF.Sqrt, bias=eps_c)
                nc.vector.reciprocal(ss, ss)
                ksc = pool_tmp.tile([128, 8], f32, tag="nr_ksc")
                nc.vector.tensor_mul(ksc, in0=ss,
                                     in1=sqrtsc_rep[:, ch * 8:(ch + 1) * 8])
                kb = pool_tmp.tile([128, 512], bf16, tag="nr_kb")
                kbv = kb.rearrange("p (h d) -> p h d", h=8)
                nc.vector.tensor_mul(
                    kbv, in0=knv,
                    in1=ksc.unsqueeze(2).broadcast_to([128, 8, 64]),
                )
                cosv = cos_t.rearrange("p (h t) -> p h t", h=16)[:, ch * 8:(ch + 1) * 8, :]
                sinv = sin_t.rearrange("p (h t) -> p h t", h=16)[:, ch * 8:(ch + 1) * 8, :]
                x1 = kbv[:, :, 0:32]
                x2 = kbv[:, :, 32:64]
                rv = rot_out.rearrange("p (h d) -> p h d", h=8)
                t1 = pool_tmp.tile([128, 256], bf16, tag="nr_t1")
                t2 = pool_tmp.tile([128, 256], bf16, tag="nr_t2")
                t1v = t1.rearrange("p (h t) -> p h t", h=8)
                t2v = t2.rearrange("p (h t) -> p h t", h=8)
                nc.vector.tensor_mul(t1v, in0=x1, in1=cosv)
                nc.vector.tensor_mul(t2v, in0=x2, in1=sinv)
                nc.vector.tensor_sub(rv[:, :, 0:32], in0=t1v, in1=t2v)
                nc.vector.tensor_mul(t1v, in0=x2, in1=cosv)
                nc.vector.tensor_mul(t2v, in0=x1, in1=sinv)
                nc.vector.tensor_add(rv[:, :, 32:64], in0=t1v, in1=t2v)

            def transpose_to(src_bf16, jj_slice, dst_ap, copy_engine):
                """PE-transpose src [128,128] bf16 slice into dst."""
                pt_ps = ptrans.tile([128, 128], bf16, tag="ptrans")
                nc.tensor.transpose(pt_ps, src_bf16[:, jj_slice], ident)
                if copy_engine is nc.scalar:
                    nc.scalar.copy(out=dst_ap, in_=pt_ps)
                else:
                    copy_engine.tensor_copy(out=dst_ap, in_=pt_ps)

            # ============ phases 1-4 ============
            with (
                tc.tile_pool(name="kT", bufs=8) as kTp,
                tc.tile_pool(name="vaug", bufs=16) as vaugp,
                tc.tile_pool(name="qT", bufs=8) as qTp,
            ):
                kT = [kTp.tile([128, L], bf16, tag="kT", name=f"kT{i}")
                      for i in range(8)]
                vaug = [vaugp.tile([128, 16, 65], bf16, tag="vaug", name=f"vaug{i}")
                        for i in range(NT_KV)]
                qT = [qTp.tile([128, CH], bf16, tag="qT", name=f"qTt{i}")
                      for i in range(8)]
                for i in range(NT_KV):
                    nc.gpsimd.memset(vaug[i][:, :, 64:65], 1.0)

                with tc.tile_pool(name="ph12", bufs=2) as tmp, \
                     tc.tile_pool(name="hqT", bufs=8) as hqTp, \
                     tc.tile_pool(name="kTloc", bufs=8) as kTlp, \
                     tc.tile_pool(name="vloc", bufs=4) as vlp, \
                     tc.tile_pool(name="sincos", bufs=8) as scp, \
                     tc.tile_pool(name="knat", bufs=3) as knatp, \
                     tc.tile_pool(name="nrtmp", bufs=2) as nrtmp:

                    # ---- phase 1: own-chunk rms norm + transposes ----
                    hqT = [hqTp.tile([128, CH], bf16, tag="hqT", name=f"hqT{i}")
                           for i in range(8)]
                    kT_loc = [kTlp.tile([128, CH], bf16, tag="kTloc", name=f"kTl{i}")
                              for i in range(8)]
                    v_loc = [vlp.tile([128, D], bf16, tag="vloc", name=f"vloc{i}")
                             for i in range(NT_Q)]
                    qcossin = []
                    hbs = []
                    for qi in range(NT_Q):
                        xa = tmp.tile([128, D], f32, tag="xa")
                        nc.sync.dma_start(out=xa, in_=x_q[qi * 128:(qi + 1) * 128, :])
                        hbs.append(rms_norm_bf16(tmp, xa))
                    for qi in range(NT_Q):
                        qcossin.append(sincos(scp, tmp, pos_q, qi))
                    for qi in range(NT_Q):
                        for g in range(8):
                            transpose_to(hbs[qi], slice(g * 128, (g + 1) * 128),
                                         hqT[g][:, qi * 128:(qi + 1) * 128],
                                         nc.scalar)

                    # ---- phase 2a: k and v projections first ----
                    def proj(qi, col0, ps_tag="pmm"):
                        ps = pmm.tile([128, 512], f32, tag=ps_tag)
                        for k8 in range(8):
                            nc.tensor.matmul(
                                ps,
                                lhsT=hqT[k8][:, qi * 128:(qi + 1) * 128],
                                rhs=wq[k8][:, col0:col0 + 512],
                                start=(k8 == 0), stop=(k8 == 7),
                            )
                        return ps

                    for qi in range(NT_Q if _PH >= 2 else 0):
                        cos_t, sin_t = qcossin[qi]
                        for ch in range(2):
                            ps2 = proj(qi, D + ch * 512)
                            krot = knatp.tile([128, 512], bf16, tag="qrot")
                            normrope(nrtmp, ps2, ch, cos_t, sin_t, krot)
                            for jj in range(4):
                                transpose_to(krot, slice(jj * 128, (jj + 1) * 128),
                                             kT_loc[ch * 4 + jj][:, qi * 128:(qi + 1) * 128],
                                             nc.vector)
                    if _PH >= 3:
                        for g in range(8):
                            nc.sync.dma_start(
                                out=cc_k_in[g * 64:(g + 1) * 64, :]
                                    .rearrange("a (b c) -> (a b) c", b=2),
                                in_=kT_loc[g])
                        nc.gpsimd.collective_compute(
                            "AllGather", mybir.AluOpType.bypass, replica_groups=RG,
                            ins=[cc_k_in[:].opt()], outs=[cc_k_out[:].opt()])

                    for qi in range(NT_Q if _PH >= 2 else 0):
                        for ch in range(2):
                            ps3 = proj(qi, 2 * D + ch * 512)
                            nc.vector.tensor_copy(
                                out=v_loc[qi][:, ch * 512:(ch + 1) * 512], in_=ps3)
                    if _PH >= 3:
                        for qi in range(NT_Q):
                            nc.sync.dma_start(
                                out=cc_v_in[qi * 128:(qi + 1) * 128, :],
                                in_=v_loc[qi])
                        nc.gpsimd.collective_compute(
                            "AllGather", mybir.AluOpType.bypass, replica_groups=RG,
                            ins=[cc_v_in[:].opt()], outs=[cc_v_out[:].opt()])

                    # ---- phase 2b: q projections (overlap the AGs) ----
                    for qi in range(NT_Q if _PH >= 2 else 0):
                        cos_t, sin_t = qcossin[qi]
                        for ch in range(2):
                            ps = proj(qi, ch * 512)
                            qrot = knatp.tile([128, 512], bf16, tag="qrot")
                            normrope(nrtmp, ps, ch, cos_t, sin_t, qrot)
                            for jj in range(4):
                                transpose_to(qrot, slice(jj * 128, (jj + 1) * 128),
                                             qT[ch * 4 + jj][:, qi * 128:(qi + 1) * 128],
                                             nc.vector)

                    # ---- phase 3b: unpack gathered k/v ----
                    if _PH >= 3:
                        for g in range(8):
                            for rr in range(4):
                                nc.gpsimd.dma_start(
                                    out=kT[g][:, rr * 512:(rr + 1) * 512],
                                    in_=cc_k_out[rr * 512 + g * 64:
                                                 rr * 512 + (g + 1) * 64, :]
                                        .rearrange("a (b c) -> (a b) c", b=2))
                        for i in range(NT_KV):
                            rr, il = i // 4, i % 4
                            nc.gpsimd.dma_start(
                                out=vaug[i][:, :, 0:64],
                                in_=cc_v_out[rr * 512 + il * 128:
                                             rr * 512 + (il + 1) * 128, :]
                                    .rearrange("p (h d) -> p h d", h=16))

                # ---- phase 4: attention per head ----
                with tc.tile_pool(name="PT", bufs=48) as PTp, \
                     tc.tile_pool(name="attmp", bufs=4) as attmp:
                    for h in range(NH if _PH >= 4 else 0):
                        j, rb = h // 2, 64 * (h % 2)
                        pts = []
                        for ki in range(NT_KV):
                            st = pmm.tile([128, 512], f32, tag="pmm")
                            nc.tensor.matmul(
                                st,
                                lhsT=kT[j][rb:rb + 64, ki * 128:(ki + 1) * 128],
                                rhs=qT[j][rb:rb + 64, :],
                                start=True, stop=True,
                            )
                            pt_sb = PTp.tile([128, 512], bf16, tag="PT")
                            nc.scalar.activation(pt_sb, st, AF.Exp)
                            pts.append(pt_sb)
                        o_ps = pvp.tile([65, 512], f32, tag="pv")
                        for ki in range(NT_KV):
                            nc.tensor.matmul(
                                o_ps,
                                lhsT=vaug[ki][:, h, :],
                                rhs=pts[ki],
                                start=(ki == 0), stop=(ki == NT_KV - 1),
                            )
                        dinv = attmp.tile([1, 512], f32, tag="dinv")
                        nc.vector.reciprocal(dinv, o_ps[64:65, :])
                        drep = attmp.tile([64, 512], f32, tag="drep")
                        nc.gpsimd.partition_broadcast(drep, dinv)
                        nc.vector.tensor_mul(
                            oT[h // 2][64 * (h % 2):64 * (h % 2) + 64, :],
                            in0=o_ps[0:64, :], in1=drep)

            # ============ phases 5-6 ============
            with (
                tc.tile_pool(name="x2", bufs=4) as x2p,
                tc.tile_pool(name="h2T", bufs=8) as h2Tp,
            ):
                x2 = [x2p.tile([128, D], f32, tag="x2", name=f"x2_{i}")
                      for i in range(NT_Q)]
                h2T = [h2Tp.tile([128, CH], bf16, tag="h2T", name=f"h2T{i}")
                       for i in range(8)]

                with tc.tile_pool(name="wo", bufs=16) as wop, \
                     tc.tile_pool(name="ph5", bufs=3) as ph5:
                    wo = [wop.tile([128, D], bf16, tag="wo", name=f"wo{i}")
                          for i in range(8)]
                    for hh in range(8):
                        nc.sync.dma_start(out=wo[hh],
                                          in_=woT[hh * 128:(hh + 1) * 128, :])
                    for qi in range(NT_Q if _PH >= 5 else 0):
                        xq = ph5.tile([128, D], f32, tag="xq")
                        nc.sync.dma_start(out=xq, in_=x_q[qi * 128:(qi + 1) * 128, :])
                        for ch in range(2):
                            ps = pmm.tile([128, 512], f32, tag="pmm")
                            for j2 in range(8):
                                nc.tensor.matmul(
                                    ps,
                                    lhsT=oT[j2][:, qi * 128:(qi + 1) * 128],
                                    rhs=wo[j2][:, ch * 512:(ch + 1) * 512],
                                    start=(j2 == 0), stop=(j2 == 7),
                                )
                            nc.vector.tensor_add(
                                x2[qi][:, ch * 512:(ch + 1) * 512],
                                in0=ps, in1=xq[:, ch * 512:(ch + 1) * 512])
                        # norm2 + transpose
                        h2b = rms_norm_bf16(ph5, x2[qi])
                        for g in range(8):
                            transpose_to(h2b, slice(g * 128, (g + 1) * 128),
                                         h2T[g][:, qi * 128:(qi + 1) * 128],
                                         nc.scalar)

                with tc.tile_pool(name="wdn", bufs=24) as wdnp, \
                     tc.tile_pool(name="aT", bufs=24) as aTp, \
                     tc.tile_pool(name="hfT", bufs=24) as hfTp, \
                     tc.tile_pool(name="ffntmp", bufs=2) as ffntmp:
                    wdn = [wdnp.tile([128, D], bf16, tag="wdn", name=f"wdn{i}")
                           for i in range(24)]
                    for jt in range(24 if _PH >= 6 else 0):
                        nc.sync.dma_start(
                            out=wdn[jt], in_=wdownT[jt * 128:(jt + 1) * 128, :])

                    # up-proj a-half
                    wua = [w3072.tile([128, 3 * D], bf16, tag="w3072", name=f"wua{i}")
                           for i in range(8)]
                    for k8 in range(8 if _PH >= 6 else 0):
                        nc.sync.dma_start(out=wua[k8],
                                          in_=wupT[k8 * 128:(k8 + 1) * 128, 0:DFF])
                    aT = [aTp.tile([128, CH], bf16, tag="aT", name=f"aT{i}")
                          for i in range(24)]
                    for jt in range(24 if _PH >= 6 else 0):
                        ps = pmm.tile([128, 512], f32, tag="pmm")
                        for k8 in range(8):
                            nc.tensor.matmul(
                                ps,
                                lhsT=wua[k8][:, jt * 128:(jt + 1) * 128],
                                rhs=h2T[k8],
                                start=(k8 == 0), stop=(k8 == 7),
                            )
                        nc.scalar.copy(out=aT[jt], in_=ps)

                    # up-proj g-half + swiglu
                    wug = [w3072.tile([128, 3 * D], bf16, tag="w3072", name=f"wug{i}")
                           for i in range(8)]
                    for k8 in range(8 if _PH >= 6 else 0):
                        nc.sync.dma_start(out=wug[k8],
                                          in_=wupT[k8 * 128:(k8 + 1) * 128, DFF:2 * DFF])
                    hfT = [hfTp.tile([128, CH], bf16, tag="hfT", name=f"hfT{i}")
                           for i in range(24)]
                    for jt in range(24 if _PH >= 6 else 0):
                        ps = pmm.tile([128, 512], f32, tag="pmm")
                        for k8 in range(8):
                            nc.tensor.matmul(
                                ps,
                                lhsT=wug[k8][:, jt * 128:(jt + 1) * 128],
                                rhs=h2T[k8],
                                start=(k8 == 0), stop=(k8 == 7),
                            )
                        gs = ffntmp.tile([128, 512], bf16, tag="gs")
                        nc.scalar.activation(gs, ps, AF.Sigmoid)
                        ag = ffntmp.tile([128, 512], bf16, tag="ag")
                        nc.vector.tensor_mul(ag, in0=aT[jt], in1=ps)
                        nc.vector.tensor_mul(hfT[jt], in0=ag, in1=gs)

                    # down-proj + residual + store
                    for qi in range(NT_Q if _PH >= 6 else 0):
                        yo = ffntmp.tile([128, D], f32, tag="yo")
                        for ch in range(2):
                            ps = pmm.tile([128, 512], f32, tag="pmm")
                            for jt in range(24):
                                nc.tensor.matmul(
                                    ps,
                                    lhsT=hfT[jt][:, qi * 128:(qi + 1) * 128],
                                    rhs=wdn[jt][:, ch * 512:(ch + 1) * 512],
                                    start=(jt == 0), stop=(jt == 23),
                                )
                            nc.vector.tensor_add(
                                yo[:, ch * 512:(ch + 1) * 512],
                                in0=ps, in1=x2[qi][:, ch * 512:(ch + 1) * 512])
                        nc.sync.dma_start(out=y[qi * 128:(qi + 1) * 128, :], in_=yo)
                    if _PH < 6:
                        for qi in range(NT_Q):
                            dum = ffntmp.tile([128, D], f32, tag="yo", name=f"dum{qi}")
                            nc.vector.memset(dum, 0.0)
                            nc.sync.dma_start(out=y[qi * 128:(qi + 1) * 128, :], in_=dum)

    nc.compile()
    return nc


def _prep_inputs(inputs):
    import ml_dtypes

    bf = ml_dtypes.bfloat16
    f32 = np.float32
    x = np.asarray(inputs["x"], f32)
    pos = np.asarray(inputs["pos"], f32)
    n1 = np.asarray(inputs["norm1_scale"], f32)
    n2 = np.asarray(inputs["norm2_scale"], f32)
    qkv_w = np.asarray(inputs["qkv_w"], f32)
    attn_scale = np.asarray(inputs["attn_scale"], f32)
    freqs = np.asarray(inputs["freqs"], f32)
    out_w = np.asarray(inputs["out_w"], f32)
    up_w = np.asarray(inputs["up_w"], f32)
    down_w = np.asarray(inputs["down_w"], f32)

    shared = {
        "wqkvT": np.ascontiguousarray((qkv_w * n1[None, :]).T).astype(bf),
        "woT": np.ascontiguousarray(out_w.T).astype(bf),
        "wupT": np.ascontiguousarray((up_w * n2[None, :]).T).astype(bf),
        "wdownT": np.ascontiguousarray(down_w.T).astype(bf),
        "freqs_c": np.ascontiguousarray(freqs.reshape(1, 256)).astype(f32),
        "sqrtsc_c": np.sqrt(attn_scale).reshape(1, 16).astype(f32),
    }
    in_maps = []
    for core in range(N_CORES):
        b, c = core // 4, core % 4
        m = dict(shared)
        m["x_q"] = np.ascontiguousarray(x[b, c * CH:(c + 1) * CH])
        m["pos_q"] = np.ascontiguousarray(pos[b, c * CH:(c + 1) * CH])
        in_maps.append(m)
    return in_maps


LAST_EXEC_NS = None


def kernel(**inputs):
    global LAST_EXEC_NS
    from concourse.bass_utils import run_bass_kernel_spmd

    if "nc" not in _CACHE:
        _CACHE["nc"] = _build_nc()
    nc = _CACHE["nc"]
    in_maps = _prep_inputs(inputs)
    trace = bool(os.environ.get("BASS_KERNEL_TRACE"))
    res = run_bass_kernel_spmd(nc, in_maps, core_ids=list(range(N_CORES)),
                               trace=trace)
    LAST_EXEC_NS = res.exec_time_ns
    out = np.empty((2, L, D), np.float32)
    for core in range(N_CORES):
        b, c = core // 4, core % 4
        out[b, c * CH:(c + 1) * CH] = res.results[core]["y"]
    return out


# revision 19
# speedup vs baseline: 1.2106x; 1.2106x over previous
"""Trainium2 Bass kernel for a dense transformer layer (AxialRoPE + cosine-sim
attention + SwiGLU FFN), distributed over 8 NeuronCores.

Sharding: core = 4*b + c handles batch b (of 2) and query-row chunk c (of 4,
512 rows each). Each core computes q/k/v for its own 512 rows; k (transposed,
rope'd, bf16) and v are AllGathered across the 4-core batch group so each core
attends over the full 2048-token sequence.

All matmuls in bf16; softmax/norms in f32.
"""

import math
import os

import numpy as np

D = 1024
L = 2048
NH = 16
DH = 64
DFF = 3072
EPS = 1e-6
CH = 512           # query rows per core
NT_KV = L // 128   # 16 l-tiles of gathered k/v
NT_Q = CH // 128   # 4 l-tiles of the own chunk
N_CORES = 8
RG = [[0, 1, 2, 3], [4, 5, 6, 7]]

_CACHE = {}
_PH = int(os.environ.get("BASS_KERNEL_PHASES", "6"))


def _build_nc():
    import concourse.bass as bass
    import concourse.mybir as mybir
    import concourse.tile as tile
    from concourse import bacc
    from concourse.masks import make_identity

    f32 = mybir.dt.float32
    bf16 = mybir.dt.bfloat16
    AF = mybir.ActivationFunctionType
    AX = mybir.AxisListType

    nc = bacc.Bacc("TRN2", target_bir_lowering=False, debug=False,
                   num_devices=N_CORES)

    x_q = nc.dram_tensor("x_q", [CH, D], f32, kind="ExternalInput").ap()
    pos_q = nc.dram_tensor("pos_q", [CH, 2], f32, kind="ExternalInput").ap()
    wqkvT = nc.dram_tensor("wqkvT", [D, 3 * D], bf16, kind="ExternalInput").ap()
    woT = nc.dram_tensor("woT", [D, D], bf16, kind="ExternalInput").ap()
    wupT = nc.dram_tensor("wupT", [D, 2 * DFF], bf16, kind="ExternalInput").ap()
    wdownT = nc.dram_tensor("wdownT", [DFF, D], bf16, kind="ExternalInput").ap()
    freqs_c = nc.dram_tensor("freqs_c", [1, 256], f32, kind="ExternalInput").ap()
    sqrtsc_c = nc.dram_tensor("sqrtsc_c", [1, 16], f32, kind="ExternalInput").ap()
    y = nc.dram_tensor("y", [CH, D], f32, kind="ExternalOutput").ap()

    # collective bounce buffers (internal DRAM); k and v gathered separately
    # so the k AllGather launches early and overlaps the rest of phase 2
    cc_k_in = nc.dram_tensor("cc_k_in", [512, D], bf16).ap()
    cc_k_out = nc.dram_tensor("cc_k_out", [2048, D], bf16).ap()
    cc_v_in = nc.dram_tensor("cc_v_in", [512, D], bf16).ap()
    cc_v_out = nc.dram_tensor("cc_v_out", [2048, D], bf16).ap()

    with tile.TileContext(nc) as tc:
        with (
            tc.tile_pool(name="const", bufs=1) as constp,
            tc.tile_pool(name="w3072", bufs=8) as w3072,
            tc.tile_pool(name="oT", bufs=16) as oTp,
            tc.tile_pool(name="ptrans", bufs=2, space="PSUM") as ptrans,
            tc.tile_pool(name="pmm", bufs=4, space="PSUM") as pmm,
            tc.tile_pool(name="pv", bufs=2, space="PSUM") as pvp,
        ):
            ident = constp.tile([128, 128], bf16)
            make_identity(nc, ident)
            freqs_rep = constp.tile([128, 256], f32)
            nc.sync.dma_start(out=freqs_rep, in_=freqs_c.broadcast_to([128, 256]))
            sqrtsc_rep = constp.tile([128, 16], f32)
            nc.sync.dma_start(out=sqrtsc_rep, in_=sqrtsc_c.broadcast_to([128, 16]))
            eps_c = constp.tile([128, 1], f32)
            nc.gpsimd.memset(eps_c, EPS)

            # qkv weights resident for phase 2
            wq = [w3072.tile([128, 3 * D], bf16, tag="w3072", name=f"wq{i}")
                  for i in range(8)]
            for k8 in range(8):
                nc.sync.dma_start(out=wq[k8], in_=wqkvT[k8 * 128:(k8 + 1) * 128, :])

            # attention output (transposed), one base-0 tile per head
            oT = [oTp.tile([128, CH], bf16, tag="oT", name=f"oT{i}")
                  for i in range(8)]

            def sincos(pool_sc, pool_tmp, pos_ap, i):
                """cos/sin tiles [128, 512] (16 heads x 32) for l-tile i."""
                pt = pool_tmp.tile([128, 2], f32, tag="pos")
                nc.sync.dma_start(out=pt, in_=pos_ap[i * 128:(i + 1) * 128, :])
                theta = pool_tmp.tile([128, 512], f32, tag="theta")
                tv = theta.rearrange("p (h t) -> p h t", h=16)
                fv = freqs_rep.rearrange("p (h t) -> p h t", h=16)
                nc.vector.tensor_scalar_mul(tv[:, :, 0:16], in0=fv, scalar1=pt[:, 0:1])
                nc.vector.tensor_scalar_mul(tv[:, :, 16:32], in0=fv, scalar1=pt[:, 1:2])
                # range-reduce into [-pi, pi] (ACT Sin domain):
                # k ~ theta/2pi (any rounding), theta -= k*2pi, one +-2pi wrap
                thk = pool_tmp.tile([128, 512], f32, tag="thk")
                nc.vector.tensor_scalar_mul(thk, in0=theta,
                                            scalar1=1.0 / (2 * math.pi))
                ki32 = pool_tmp.tile([128, 512], mybir.dt.int32, tag="thk")
                nc.vector.tensor_copy(out=ki32, in_=thk)
                kf = pool_tmp.tile([128, 512], f32, tag="thk")
                nc.vector.tensor_copy(out=kf, in_=ki32)
                c1 = float(np.float32(2 * math.pi))
                c2 = float(2 * math.pi - np.float64(np.float32(2 * math.pi)))
                nc.vector.cody_waite_cascade(out=theta, x=theta, k=kf,
                                             c1=c1, c2=c2, c3=0.0)
                nc.vector.add_range_wrap(out=theta, in_=theta, shift=0.0,
                                         bound=math.pi, period=2 * math.pi)
                cos_t = pool_sc.tile([128, 512], bf16, tag="sincos")
                sin_t = pool_sc.tile([128, 512], bf16, tag="sincos")
                nc.scalar.activation(sin_t, theta, AF.Sin)
                nc.vector.add_range_wrap(out=theta, in_=theta, shift=math.pi / 2,
                                         bound=math.pi, period=2 * math.pi)
                nc.scalar.activation(cos_t, theta, AF.Sin)
                return cos_t, sin_t

            def rms_norm_bf16(pool_tmp, xa):
                """xa [128, 1024] f32 -> h bf16 [128, 1024]."""
                sq = pool_tmp.tile([128, D], bf16, tag="sq")
                nc.vector.tensor_mul(sq, in0=xa, in1=xa)
                ssum = pool_tmp.tile([128, 1], f32, tag="ssum")
                nc.vector.reduce_sum(out=ssum, in_=sq, axis=AX.X)
                rstd = pool_tmp.tile([128, 1], f32, tag="rstd")
                nc.scalar.activation(rstd, ssum, AF.Sqrt, bias=eps_c, scale=1.0 / D)
                nc.vector.reciprocal(rstd, rstd)
                hb = pool_tmp.tile([128, D], bf16, tag="hb")
                nc.vector.tensor_scalar_mul(hb, in0=xa, scalar1=rstd)
                return hb

            def normrope(pool_tmp, kn, ch, cos_t, sin_t, rot_out):
                """cosine-normalize + rope heads ch*8..ch*8+8.

                kn: [128, 512] f32 PSUM (8 heads x 64); rot_out bf16 SBUF."""
                knv = kn.rearrange("p (h d) -> p h d", h=8)
                sq = pool_tmp.tile([128, 512], bf16, tag="nr_sq")
                sqv = sq.rearrange("p (h d) -> p h d", h=8)
                nc.scalar.square(sq, kn)
                ss = pool_tmp.tile([128, 8], f32, tag="nr_ss")
                nc.vector.reduce_sum(out=ss, in_=sqv, axis=AX.X)
                nc.scalar.activation(ss, ss, AF.Sqrt, bias=eps_c)
                nc.vector.reciprocal(ss, ss)
                ksc = pool_tmp.tile([128, 8], f32, tag="nr_ksc")
                nc.vector.tensor_mul(ksc, in0=ss,
                                     in1=sqrtsc_rep[:, ch * 8:(ch + 1) * 8])
                kb = pool_tmp.tile([128, 512], bf16, tag="nr_kb")
                kbv = kb.rearrange("p (h d) -> p h d", h=8)
                nc.vector.tensor_mul(
                    kbv, in0=knv,
                    in1=ksc.unsqueeze(2).broadcast_to([128, 8, 64]),
                )
                cosv = cos_t.rearrange("p (h t) -> p h t", h=16)[:, ch * 8:(ch + 1) * 8, :]
                sinv = sin_t.rearrange("p (h t) -> p h t", h=16)[:, ch * 8:(ch + 1) * 8, :]
                x1 = kbv[:, :, 0:32]
                x2 = kbv[:, :, 32:64]
                rv = rot_out.rearrange("p (h d) -> p h d", h=8)
                t1 = pool_tmp.tile([128, 256], bf16, tag="nr_t1")
                t2 = pool_tmp.tile([128, 256], bf16, tag="nr_t2")
                t1v = t1.rearrange("p (h t) -> p h t", h=8)
                t2v = t2.rearrange("p (h t) -> p h t", h=8)
                nc.vector.tensor_mul(t1v, in0=x1, in1=cosv)
                nc.vector.tensor_mul(t2v, in0=x2, in1=sinv)
                nc.vector.tensor_sub(rv[:, :, 0:32], in0=t1v, in1=t2v)
                nc.vector.tensor_mul(t1v, in0=x2, in1=cosv)
                nc.vector.tensor_mul(t2v, in0=x1, in1=sinv)
                nc.vector.tensor_add(rv[:, :, 32:64], in0=t1v, in1=t2v)

            def transpose_to(src_bf16, jj_slice, dst_ap, copy_engine):
                """PE-transpose src [128,128] bf16 slice into dst."""
                pt_ps = ptrans.tile([128, 128], bf16, tag="ptrans")
                nc.tensor.transpose(pt_ps, src_bf16[:, jj_slice], ident)
                if copy_engine is nc.scalar:
                    nc.scalar.copy(out=dst_ap, in_=pt_ps)
                else:
                    copy_engine.tensor_copy(out=dst_ap, in_=pt_ps)

            # ============ phases 1-4 ============
            with (
                tc.tile_pool(name="kT", bufs=8) as kTp,
                tc.tile_pool(name="vaug", bufs=16) as vaugp,
                tc.tile_pool(name="qT", bufs=8) as qTp,
            ):
                kT = [kTp.tile([128, L], bf16, tag="kT", name=f"kT{i}")
                      for i in range(8)]
                vaug = [vaugp.tile([128, 16, 65], bf16, tag="vaug", name=f"vaug{i}")
                        for i in range(NT_KV)]
                qT = [qTp.tile([128, CH], bf16, tag="qT", name=f"qTt{i}")
                      for i in range(8)]
                for i in range(NT_KV):
                    nc.gpsimd.memset(vaug[i][:, :, 64:65], 1.0)

                with tc.tile_pool(name="ph12", bufs=2) as tmp, \
                     tc.tile_pool(name="hqT", bufs=8) as hqTp, \
                     tc.tile_pool(name="kTloc", bufs=8) as kTlp, \
                     tc.tile_pool(name="vloc", bufs=4) as vlp, \
                     tc.tile_pool(name="sincos", bufs=8) as scp, \
                     tc.tile_pool(name="knat", bufs=3) as knatp, \
                     tc.tile_pool(name="nrtmp", bufs=2) as nrtmp:

                    # ---- phase 1: own-chunk rms norm + transposes ----
                    hqT = [hqTp.tile([128, CH], bf16, tag="hqT", name=f"hqT{i}")
                           for i in range(8)]
                    kT_loc = [kTlp.tile([128, CH], bf16, tag="kTloc", name=f"kTl{i}")
                              for i in range(8)]
                    v_loc = [vlp.tile([128, D], bf16, tag="vloc", name=f"vloc{i}")
                             for i in range(NT_Q)]
                    qcossin = []
                    hbs = []
                    for qi in range(NT_Q):
                        xa = tmp.tile([128, D], f32, tag="xa")
                        nc.sync.dma_start(out=xa, in_=x_q[qi * 128:(qi + 1) * 128, :])
                        hbs.append(rms_norm_bf16(tmp, xa))
                    for qi in range(NT_Q):
                        qcossin.append(sincos(scp, tmp, pos_q, qi))
                    for qi in range(NT_Q):
                        for g in range(8):
                            transpose_to(hbs[qi], slice(g * 128, (g + 1) * 128),
                                         hqT[g][:, qi * 128:(qi + 1) * 128],
                                         nc.scalar)

                    # ---- phase 2a: k and v projections first ----
                    def proj(qi, col0, ps_tag="pmm"):
                        ps = pmm.tile([128, 512], f32, tag=ps_tag)
                        for k8 in range(8):
                            nc.tensor.matmul(
                                ps,
                                lhsT=hqT[k8][:, qi * 128:(qi + 1) * 128],
                                rhs=wq[k8][:, col0:col0 + 512],
                                start=(k8 == 0), stop=(k8 == 7),
                            )
                        return ps

                    for qi in range(NT_Q if _PH >= 2 else 0):
                        cos_t, sin_t = qcossin[qi]
                        for ch in range(2):
                            ps2 = proj(qi, D + ch * 512)
                            krot = knatp.tile([128, 512], bf16, tag="qrot")
                            normrope(nrtmp, ps2, ch, cos_t, sin_t, krot)
                            for jj in range(4):
                                transpose_to(krot, slice(jj * 128, (jj + 1) * 128),
                                             kT_loc[ch * 4 + jj][:, qi * 128:(qi + 1) * 128],
                                             nc.vector)
                    if _PH >= 3:
                        for g in range(8):
                            nc.sync.dma_start(
                                out=cc_k_in[g * 64:(g + 1) * 64, :]
                                    .rearrange("a (b c) -> (a b) c", b=2),
                                in_=kT_loc[g])
                        nc.gpsimd.collective_compute(
                            "AllGather", mybir.AluOpType.bypass, replica_groups=RG,
                            ins=[cc_k_in[:].opt()], outs=[cc_k_out[:].opt()])

                    for qi in range(NT_Q if _PH >= 2 else 0):
                        for ch in range(2):
                            ps3 = proj(qi, 2 * D + ch * 512)
                            nc.vector.tensor_copy(
                                out=v_loc[qi][:, ch * 512:(ch + 1) * 512], in_=ps3)
                    if _PH >= 3:
                        for qi in range(NT_Q):
                            nc.sync.dma_start(
                                out=cc_v_in[qi * 128:(qi + 1) * 128, :],
                                in_=v_loc[qi])
                        nc.gpsimd.collective_compute(
                            "AllGather", mybir.AluOpType.bypass, replica_groups=RG,
                            ins=[cc_v_in[:].opt()], outs=[cc_v_out[:].opt()])

                    # ---- phase 2b: q projections (overlap the AGs) ----
                    for qi in range(NT_Q if _PH >= 2 else 0):
                        cos_t, sin_t = qcossin[qi]
                        for ch in range(2):
                            ps = proj(qi, ch * 512)
                            qrot = knatp.tile([128, 512], bf16, tag="qrot")
                            normrope(nrtmp, ps, ch, cos_t, sin_t, qrot)
                            for jj in range(4):
                                transpose_to(qrot, slice(jj * 128, (jj + 1) * 128),
                                             qT[ch * 4 + jj][:, qi * 128:(qi + 1) * 128],
                                             nc.vector)

                    # ---- phase 3b: unpack gathered k/v ----
                    if _PH >= 3:
                        for g in range(8):
                            for rr in range(4):
                                nc.gpsimd.dma_start(
                                    out=kT[g][:, rr * 512:(rr + 1) * 512],
                                    in_=cc_k_out[rr * 512 + g * 64:
                                                 rr * 512 + (g + 1) * 64, :]
                                        .rearrange("a (b c) -> (a b) c", b=2))
                        for i in range(NT_KV):
                            rr, il = i // 4, i % 4
                            nc.gpsimd.dma_start(
                                out=vaug[i][:, :, 0:64],
                                in_=cc_v_out[rr * 512 + il * 128:
                                             rr * 512 + (il + 1) * 128, :]
                                    .rearrange("p (h d) -> p h d", h=16))

                # ---- phase 4: attention per head ----
                with tc.tile_pool(name="PT", bufs=48) as PTp, \
                     tc.tile_pool(name="attmp", bufs=4) as attmp:
                    for h in range(NH if _PH >= 4 else 0):
                        j, rb = h // 2, 64 * (h % 2)
                        pts = []
                        for ki in range(NT_KV):
                            st = pmm.tile([128, 512], f32, tag="pmm")
                            nc.tensor.matmul(
                                st,
                                lhsT=kT[j][rb:rb + 64, ki * 128:(ki + 1) * 128],
                                rhs=qT[j][rb:rb + 64, :],
                                start=True, stop=True,
                            )
                            pt_sb = PTp.tile([128, 512], bf16, tag="PT")
                            nc.scalar.activation(pt_sb, st, AF.Exp)
                            pts.append(pt_sb)
                        o_ps = pvp.tile([65, 512], f32, tag="pv")
                        for ki in range(NT_KV):
                            nc.tensor.matmul(
                                o_ps,
                                lhsT=vaug[ki][:, h, :],
                                rhs=pts[ki],
                                start=(ki == 0), stop=(ki == NT_KV - 1),
                            )
                        dinv = attmp.tile([1, 512], f32, tag="dinv")
                        nc.vector.reciprocal(dinv, o_ps[64:65, :])
                        drep = attmp.tile([64, 512], f32, tag="drep")
                        nc.gpsimd.partition_broadcast(drep, dinv)
                        nc.vector.tensor_mul(
                            oT[h // 2][64 * (h % 2):64 * (h % 2) + 64, :],
                            in0=o_ps[0:64, :], in1=drep)

            # ============ phases 5-6 ============
            with (
                tc.tile_pool(name="x2", bufs=4) as x2p,
                tc.tile_pool(name="h2T", bufs=8) as h2Tp,
            ):
                x2 = [x2p.tile([128, D], f32, tag="x2", name=f"x2_{i}")
                      for i in range(NT_Q)]
                h2T = [h2Tp.tile([128, CH], bf16, tag="h2T", name=f"h2T{i}")
                       for i in range(8)]

                with tc.tile_pool(name="wo", bufs=16) as wop, \
                     tc.tile_pool(name="ph5", bufs=3) as ph5:
                    wo = [wop.tile([128, D], bf16, tag="wo", name=f"wo{i}")
                          for i in range(8)]
                    for hh in range(8):
                        nc.sync.dma_start(out=wo[hh],
                                          in_=woT[hh * 128:(hh + 1) * 128, :])
                    for qi in range(NT_Q if _PH >= 5 else 0):
                        xq = ph5.tile([128, D], f32, tag="xq")
                        nc.sync.dma_start(out=xq, in_=x_q[qi * 128:(qi + 1) * 128, :])
                        for ch in range(2):
                            ps = pmm.tile([128, 512], f32, tag="pmm")
                            for j2 in range(8):
                                nc.tensor.matmul(
                                    ps,
                                    lhsT=oT[j2][:, qi * 128:(qi + 1) * 128],
                                    rhs=wo[j2][:, ch * 512:(ch + 1) * 512],
                                    start=(j2 == 0), stop=(j2 == 7),
                                )
                            nc.vector.tensor_add(
                                x2[qi][:, ch * 512:(ch + 1) * 512],
                                in0=ps, in1=xq[:, ch * 512:(ch + 1) * 512])
                        # norm2 + transpose
                        h2b = rms_norm_bf16(ph5, x2[qi])
                        for g in range(8):
                            transpose_to(h2b, slice(g * 128, (g + 1) * 128),
                                         h2T[g][:, qi * 128:(qi + 1) * 128],
                                         nc.scalar)

                with tc.tile_pool(name="wdn", bufs=24) as wdnp, \
                     tc.tile_pool(name="aT", bufs=24) as aTp, \
                     tc.tile_pool(name="hfT", bufs=24) as hfTp, \
                     tc.tile_pool(name="ffntmp", bufs=2) as ffntmp:
                    wdn = [wdnp.tile([128, D], bf16, tag="wdn", name=f"wdn{i}")
                           for i in range(24)]
                    for jt in range(24 if _PH >= 6 else 0):
                        nc.sync.dma_start(
                            out=wdn[jt], in_=wdownT[jt * 128:(jt + 1) * 128, :])

                    # up-proj a-half
                    wua = [w3072.tile([128, 3 * D], bf16, tag="w3072", name=f"wua{i}")
                           for i in range(8)]
                    for k8 in range(8 if _PH >= 6 else 0):
                        nc.sync.dma_start(out=wua[k8],
                                          in_=wupT[k8 * 128:(k8 + 1) * 128, 0:DFF])
                    aT = [aTp.tile([128, CH], bf16, tag="aT", name=f"aT{i}")
                          for i in range(24)]
                    for jt in range(24 if _PH >= 6 else 0):
                        ps = pmm.tile([128, 512], f32, tag="pmm")
                        for k8 in range(8):
                            nc.tensor.matmul(
                                ps,
                                lhsT=wua[k8][:, jt * 128:(jt + 1) * 128],
                                rhs=h2T[k8],
                                start=(k8 == 0), stop=(k8 == 7),
                            )
                        nc.scalar.copy(out=aT[jt], in_=ps)

                    # up-proj g-half + swiglu
                    wug = [w3072.tile([128, 3 * D], bf16, tag="w3072", name=f"wug{i}")
                           for i in range(8)]
                    for k8 in range(8 if _PH >= 6 else 0):
                        nc.sync.dma_start(out=wug[k8],
                                          in_=wupT[k8 * 128:(k8 + 1) * 128, DFF:2 * DFF])
                    hfT = [hfTp.tile([128, CH], bf16, tag="hfT", name=f"hfT{i}")
                           for i in range(24)]
                    for jt in range(24 if _PH >= 6 else 0):
                        ps = pmm.tile([128, 512], f32, tag="pmm")
                        for k8 in range(8):
                            nc.tensor.matmul(
                                ps,
                                lhsT=wug[k8][:, jt * 128:(jt + 1) * 128],
                                rhs=h2T[k8],
                                start=(k8 == 0), stop=(k8 == 7),
                            )
                        gs = ffntmp.tile([128, 512], bf16, tag="gs")
                        nc.scalar.activation(gs, ps, AF.Sigmoid)
                        ag = ffntmp.tile([128, 512], bf16, tag="ag")
                        nc.vector.tensor_mul(ag, in0=aT[jt], in1=ps)
                        nc.vector.tensor_mul(hfT[jt], in0=ag, in1=gs)

                    # down-proj + residual + store
                    for qi in range(NT_Q if _PH >= 6 else 0):
                        yo = ffntmp.tile([128, D], f32, tag="yo")
                        for ch in range(2):
                            ps = pmm.tile([128, 512], f32, tag="pmm")
                            for jt in range(24):
                                nc.tensor.matmul(
                                    ps,
                                    lhsT=hfT[jt][:, qi * 128:(qi + 1) * 128],
                                    rhs=wdn[jt][:, ch * 512:(ch + 1) * 512],
                                    start=(jt == 0), stop=(jt == 23),
                                )
                            nc.vector.tensor_add(
                                yo[:, ch * 512:(ch + 1) * 512],
                                in0=ps, in1=x2[qi][:, ch * 512:(ch + 1) * 512])
                        nc.sync.dma_start(out=y[qi * 128:(qi + 1) * 128, :], in_=yo)
                    if _PH < 6:
                        for qi in range(NT_Q):
                            dum = ffntmp.tile([128, D], f32, tag="yo", name=f"dum{qi}")
                            nc.vector.memset(dum, 0.0)
                            nc.sync.dma_start(out=y[qi * 128:(qi + 1) * 128, :], in_=dum)

    nc.compile()
    return nc


def _prep_inputs(inputs):
    import ml_dtypes

    bf = ml_dtypes.bfloat16
    f32 = np.float32
    x = np.asarray(inputs["x"], f32)
    pos = np.asarray(inputs["pos"], f32)
    n1 = np.asarray(inputs["norm1_scale"], f32)
    n2 = np.asarray(inputs["norm2_scale"], f32)
    qkv_w = np.asarray(inputs["qkv_w"], f32)
    attn_scale = np.asarray(inputs["attn_scale"], f32)
    freqs = np.asarray(inputs["freqs"], f32)
    out_w = np.asarray(inputs["out_w"], f32)
    up_w = np.asarray(inputs["up_w"], f32)
    down_w = np.asarray(inputs["down_w"], f32)

    shared = {
        "wqkvT": np.ascontiguousarray((qkv_w * n1[None, :]).T).astype(bf),
        "woT": np.ascontiguousarray(out_w.T).astype(bf),
        "wupT": np.ascontiguousarray((up_w * n2[None, :]).T).astype(bf),
        "wdownT": np.ascontiguousarray(down_w.T).astype(bf),
        "freqs_c": np.ascontiguousarray(freqs.reshape(1, 256)).astype(f32),
        "sqrtsc_c": np.sqrt(attn_scale).reshape(1, 16).astype(f32),
    }
    in_maps = []
    for core in range(N_CORES):
        b, c = core // 4, core % 4
        m = dict(shared)
        m["x_q"] = np.ascontiguousarray(x[b, c * CH:(c + 1) * CH])
        m["pos_q"] = np.ascontiguousarray(pos[b, c * CH:(c + 1) * CH])
        in_maps.append(m)
    return in_maps


LAST_EXEC_NS = None


def kernel(**inputs):
    global LAST_EXEC_NS
    from concourse.bass_utils import run_bass_kernel_spmd

    if "nc" not in _CACHE:
        _CACHE["nc"] = _build_nc()
    nc = _CACHE["nc"]
    in_maps = _prep_inputs(inputs)
    trace = bool(os.environ.get("BASS_KERNEL_TRACE"))
    res = run_bass_kernel_spmd(nc, in_maps, core_ids=list(range(N_CORES)),
                               trace=trace)
    LAST_EXEC_NS = res.exec_time_ns
    out = np.empty((2, L, D), np.float32)
    for core in range(N_CORES):
        b, c = core // 4, core % 4
        out[b, c * CH:(c + 1) * CH] = res.results[core]["y"]
    return out
